# revision 5
# baseline (speedup 1.0000x reference)
"""Segment-max (GridPooling) kernel for 8 trn2 NeuronCores.

Strategy:
  Host: sort points by cell_idx, pad each segment's run to a multiple of
  W=8 with duplicate points (duplicates are max-neutral), split the
  padded stream into 16 contiguous chunks at segment boundaries
  (8 cores x 2 partition-half streams; features live on partitions:
  stream A on partitions 0-63, stream B on 64-127).
  Device (per core): stream [128, TF] tiles; grouped tensor_reduce max
  [128, G, 8] -> [128, G]; then a segmented running-max via
  tensor_tensor_scan (state = max(state + m, v), m in {0, -1e30} at run
  starts; mask broadcast from partitions {0,32,64,96} via stream_shuffle).
  Run-end columns of the scan stream hold exact per-segment maxima.
  Host: pick run-end columns per segment, transpose, concatenate.
  All reductions are f32 max -> bit-exact vs the reference.
"""
import sys

if "/opt/trn_rl_repo" not in sys.path:
    sys.path.insert(0, "/opt/trn_rl_repo")

import numpy as np

W = 8          # level-1 reduction width (pad runs to multiples of this)
TF = 8192      # raw slots per tile
NCORES = 8
NEG = np.float32(-1e30)

_nc_cache = {}


def _build_nc(l_half, g_half, ntiles):
    import concourse.bass as bass
    from concourse import mybir

    f32 = mybir.dt.float32
    nc = bass.Bass()
    x_ext = nc.declare_dram_parameter("x", [128, l_half], f32, isOutput=False)
    m_ext = nc.declare_dram_parameter("m", [4, g_half], f32, isOutput=False)
    s_ext = nc.declare_dram_parameter("scan", [128, g_half], f32, isOutput=True)

    NB = 3                     # raw-tile buffers
    GT = TF // W               # supers per tile

    import contextlib
    ctx = contextlib.ExitStack()
    with ctx:
        xt = [ctx.enter_context(nc.sbuf_tensor(f"xt{i}", [128, TF], f32)) for i in range(NB)]
        mt = [ctx.enter_context(nc.sbuf_tensor(f"mt{i}", [128, GT], f32)) for i in range(2)]
        bc = [ctx.enter_context(nc.sbuf_tensor(f"bc{i}", [128, GT], f32)) for i in range(2)]
        red = [ctx.enter_context(nc.sbuf_tensor(f"red{i}", [128, GT], f32)) for i in range(2)]
        st = [ctx.enter_context(nc.sbuf_tensor(f"st{i}", [128, GT], f32)) for i in range(2)]
        in_sems = [ctx.enter_context(nc.semaphore(f"in_sem{i}")) for i in range(NB)]
        mk_sems = [ctx.enter_context(nc.semaphore(f"mk_sem{i}")) for i in range(2)]
        out_sems = [ctx.enter_context(nc.semaphore(f"out_sem{i}")) for i in range(2)]
        v_sem = ctx.enter_context(nc.semaphore("v_sem"))
        ms_sem = ctx.enter_context(nc.semaphore("ms_sem"))
        block = ctx.enter_context(nc.Block())

        def mask_dma(s, i):
            for r in range(4):
                s.dma_start(mt[i % 2][32 * r:32 * r + 1, :],
                            m_ext[r, i * GT:(i + 1) * GT][None, :]).then_inc(mk_sems[i % 2], 16)

        @block.sync
        def _(s):
            # wait for mask-buffer memsets (partitions not covered by mask DMA)
            s.wait_ge(ms_sem, 1)
            for i in range(min(NB, ntiles)):
                s.dma_start(xt[i][:], x_ext[:, i * TF:(i + 1) * TF]).then_inc(in_sems[i], 16)
            for i in range(min(2, ntiles)):
                mask_dma(s, i)
            for i in range(ntiles):
                s.wait_ge(v_sem, i + 1)
                s.dma_start(s_ext[:, i * GT:(i + 1) * GT], st[i % 2][:]).then_inc(out_sems[i % 2], 16)
                if i + NB < ntiles:
                    j = i + NB
                    s.dma_start(xt[j % NB][:],
                                x_ext[:, j * TF:(j + 1) * TF]).then_inc(in_sems[j % NB], 16)
                if i + 2 < ntiles:
                    mask_dma(s, i + 2)

        @block.vector
        def _(v):
            v.memset(mt[0][:], 0.0)
            v.memset(mt[1][:], 0.0).then_inc(ms_sem, 1)
            for i in range(ntiles):
                v.wait_ge(mk_sems[i % 2], 64 * (i // 2 + 1))
                v.stream_shuffle(bc[i % 2][:], mt[i % 2][:], mask=[0] * 32)
                v.wait_ge(in_sems[i % NB], 16 * (i // NB + 1))
                v.tensor_reduce(
                    red[i % 2][:], xt[i % NB][:].rearrange("p (g w) -> p g w", w=W),
                    axis=mybir.AxisListType.X, op=mybir.AluOpType.max)
                if i >= 2:
                    v.wait_ge(out_sems[i % 2], 16 * (i // 2))
                v.drain()
                init = float(NEG) if i == 0 else st[(i - 1) % 2][:, GT - 1:GT]
                v.tensor_tensor_scan(
                    st[i % 2][:], bc[i % 2][:], red[i % 2][:], initial=init,
                    op0=mybir.AluOpType.add, op1=mybir.AluOpType.max,
                ).then_inc(v_sem, 1)

    return nc


def _preprocess(sig, idx, S):
    """Sort+pad on host. Returns in_maps plus the output-assembly plan."""
    N, D = sig.shape
    assert D == 64, f"kernel assumes D=64, got {D}"
    counts = np.bincount(idx, minlength=S)
    order = np.argsort(idx, kind="stable")
    pc = ((counts + W - 1) // W) * W          # padded counts (0 stays 0)
    padded_starts = np.zeros(S + 1, np.int64)
    np.cumsum(pc, out=padded_starts[1:])
    L = int(padded_starts[-1])
    cstart = np.zeros(S + 1, np.int64)
    np.cumsum(counts, out=cstart[1:])

    sid = np.repeat(np.arange(S, dtype=np.int64), pc)
    pos = np.arange(L, dtype=np.int64) - padded_starts[sid]
    src_sorted = cstart[sid] + np.minimum(pos, counts[sid] - 1)
    perm = order[src_sorted]                  # padded stream -> signal row

    # 16 chunks at segment boundaries
    targets = (L * np.arange(1, 16, dtype=np.int64)) // 16
    split_segs = np.searchsorted(padded_starts, targets, side="left")
    seg_bounds = np.concatenate([[0], split_segs, [S]])
    seg_bounds = np.maximum.accumulate(seg_bounds)  # ensure monotone
    slot_bounds = padded_starts[seg_bounds]

    lh_real = np.diff(slot_bounds)
    l_half = int(-(-int(lh_real.max()) // TF) * TF)
    g_half = l_half // W
    ntiles = l_half // TF

    sig_t = np.ascontiguousarray(sig.T)       # [64, N]
    in_maps = []
    plans = []   # per half: (core, rows_lo, s_lo, s_hi, base_slot)
    for c in range(NCORES):
        X = np.empty((128, l_half), np.float32)
        M = np.empty((4, g_half), np.float32)
        for h in range(2):
            k = 2 * c + h
            s_lo, s_hi = int(seg_bounds[k]), int(seg_bounds[k + 1])
            b0, b1 = int(slot_bounds[k]), int(slot_bounds[k + 1])
            hperm = perm[b0:b1]
            if len(hperm) < l_half:
                pad_src = hperm[-1] if len(hperm) else 0
                hperm = np.concatenate(
                    [hperm, np.full(l_half - len(hperm), pad_src, np.int64)])
            X[64 * h:64 * (h + 1), :] = sig_t[:, hperm]
            mrow = np.zeros(g_half, np.float32)
            starts_local = (padded_starts[s_lo:s_hi] - b0) // W
            mrow[starts_local[(pc[s_lo:s_hi] > 0)]] = NEG
            mrow[(b1 - b0) // W:] = NEG       # dummy tail groups: isolate
            M[2 * h:2 * (h + 1), :] = mrow[None, :]
            plans.append((c, 64 * h, s_lo, s_hi, b0))
        in_maps.append({"x": X, "m": M})
    return in_maps, plans, padded_starts, pc, l_half, g_half, ntiles


def kernel(signal, cell_idx, num_segments):
    from concourse.bass_utils import run_bass_kernel_spmd

    sig = np.asarray(signal, dtype=np.float32)
    idx = np.asarray(cell_idx).astype(np.int64)
    S = int(num_segments)

    in_maps, plans, padded_starts, pc, l_half, g_half, ntiles = _preprocess(sig, idx, S)

    key = (l_half, g_half, ntiles)
    if key not in _nc_cache:
        _nc_cache[key] = _build_nc(l_half, g_half, ntiles)
    nc = _nc_cache[key]

    res = run_bass_kernel_spmd(nc, in_maps, core_ids=list(range(NCORES)))

    out = np.full((S, sig.shape[1]), -np.inf, np.float32)
    for (c, r0, s_lo, s_hi, b0) in plans:
        if s_hi <= s_lo:
            continue
        scan = res.results[c]["scan"][r0:r0 + 64]          # [64, g_half]
        nz = pc[s_lo:s_hi] > 0
        ends_local = (padded_starts[s_lo + 1:s_hi + 1] - b0) // W - 1
        out[np.arange(s_lo, s_hi)[nz]] = scan[:, ends_local[nz]].T
    return out


# revision 6
# speedup vs baseline: 7.2589x; 7.2589x over previous
"""Segment-max (GridPooling) kernel for 8 trn2 NeuronCores.

Strategy:
  Host: sort points by cell_idx, pad each segment's run to a multiple of
  W=8 with duplicate points (duplicates are max-neutral), split the
  padded stream into 16 contiguous chunks at segment boundaries
  (8 cores x 2 partition-half streams; features live on partitions:
  stream A on partitions 0-63, stream B on 64-127).
  Device (per core): stream [128, TF] tiles; grouped tensor_reduce max
  [128, G, 8] -> [128, G]; then a segmented running-max via
  tensor_tensor_scan (state = max(state + m, v), m in {0, -1e30} at run
  starts; mask broadcast from partitions {0,32,64,96} via stream_shuffle).
  Run-end columns of the scan stream hold exact per-segment maxima.
  Host: pick run-end columns per segment, transpose, concatenate.
  All reductions are f32 max -> bit-exact vs the reference.
"""
import sys

if "/opt/trn_rl_repo" not in sys.path:
    sys.path.insert(0, "/opt/trn_rl_repo")

import numpy as np

W = 8          # level-1 reduction width (pad runs to multiples of this)
TF = 8192      # raw slots per tile
NCORES = 8
NEG = np.float32(-1e30)

_nc_cache = {}


def _build_nc(l_half, g_half, ntiles, reps=1):
    import concourse.bass as bass
    from concourse import mybir

    f32 = mybir.dt.float32
    nc = bass.Bass()
    x_ext = nc.declare_dram_parameter("x", [128, l_half], f32, isOutput=False)
    m_ext = nc.declare_dram_parameter("m", [4, g_half], f32, isOutput=False)
    s_ext = nc.declare_dram_parameter("scan", [128, g_half], f32, isOutput=True)

    NB = 3                     # raw-tile buffers
    GT = TF // W               # supers per tile

    import contextlib
    ctx = contextlib.ExitStack()
    with ctx:
        xt = [ctx.enter_context(nc.sbuf_tensor(f"xt{i}", [128, TF], f32)) for i in range(NB)]
        mt = [ctx.enter_context(nc.sbuf_tensor(f"mt{i}", [128, GT], f32)) for i in range(2)]
        bc = [ctx.enter_context(nc.sbuf_tensor(f"bc{i}", [128, GT], f32)) for i in range(2)]
        red = [ctx.enter_context(nc.sbuf_tensor(f"red{i}", [128, GT], f32)) for i in range(2)]
        st = [ctx.enter_context(nc.sbuf_tensor(f"st{i}", [128, GT], f32)) for i in range(2)]
        in_sems = [ctx.enter_context(nc.semaphore(f"in_sem{i}")) for i in range(NB)]
        mk_sems = [ctx.enter_context(nc.semaphore(f"mk_sem{i}")) for i in range(2)]
        out_sems = [ctx.enter_context(nc.semaphore(f"out_sem{i}")) for i in range(2)]
        v_sem = ctx.enter_context(nc.semaphore("v_sem"))
        ms_sem = ctx.enter_context(nc.semaphore("ms_sem"))
        block = ctx.enter_context(nc.Block())

        def mask_dma(s, i):
            d = i % ntiles
            for r in range(4):
                s.dma_start(mt[i % 2][32 * r:32 * r + 1, :],
                            m_ext[r, d * GT:(d + 1) * GT][None, :]).then_inc(mk_sems[i % 2], 16)

        total = ntiles * reps

        @block.sync
        def _(s):
            # wait for mask-buffer memsets (partitions not covered by mask DMA)
            s.wait_ge(ms_sem, 1)
            for i in range(min(NB, total)):
                d = (i % ntiles)
                s.dma_start(xt[i][:], x_ext[:, d * TF:(d + 1) * TF]).then_inc(in_sems[i], 16)
            for i in range(min(2, total)):
                mask_dma(s, i)
            for i in range(total):
                d = i % ntiles
                s.wait_ge(v_sem, i + 1)
                s.dma_start(s_ext[:, d * GT:(d + 1) * GT], st[i % 2][:]).then_inc(out_sems[i % 2], 16)
                if i + NB < total:
                    j = i + NB
                    jd = j % ntiles
                    s.dma_start(xt[j % NB][:],
                                x_ext[:, jd * TF:(jd + 1) * TF]).then_inc(in_sems[j % NB], 16)
                if i + 2 < total:
                    mask_dma(s, i + 2)

        @block.vector
        def _(v):
            v.memset(mt[0][:], 0.0)
            v.memset(mt[1][:], 0.0).then_inc(ms_sem, 1)
            for i in range(ntiles * reps):
                v.wait_ge(mk_sems[i % 2], 64 * (i // 2 + 1))
                v.stream_shuffle(bc[i % 2][:], mt[i % 2][:], mask=[0] * 32)
                v.wait_ge(in_sems[i % NB], 16 * (i // NB + 1))
                v.tensor_reduce(
                    red[i % 2][:], xt[i % NB][:].rearrange("p (g w) -> p g w", w=W),
                    axis=mybir.AxisListType.X, op=mybir.AluOpType.max)
                if i >= 2:
                    v.wait_ge(out_sems[i % 2], 16 * (i // 2))
                v.drain()
                init = float(NEG) if i == 0 else st[(i - 1) % 2][:, GT - 1:GT]
                v.tensor_tensor_scan(
                    st[i % 2][:], bc[i % 2][:], red[i % 2][:], initial=init,
                    op0=mybir.AluOpType.add, op1=mybir.AluOpType.max,
                ).then_inc(v_sem, 1)

    return nc


def _preprocess(sig, idx, S):
    """Sort+pad on host. Returns in_maps plus the output-assembly plan."""
    N, D = sig.shape
    assert D == 64, f"kernel assumes D=64, got {D}"
    counts = np.bincount(idx, minlength=S)
    order = np.argsort(idx, kind="stable")
    pc = ((counts + W - 1) // W) * W          # padded counts (0 stays 0)
    padded_starts = np.zeros(S + 1, np.int64)
    np.cumsum(pc, out=padded_starts[1:])
    L = int(padded_starts[-1])
    cstart = np.zeros(S + 1, np.int64)
    np.cumsum(counts, out=cstart[1:])

    sid = np.repeat(np.arange(S, dtype=np.int64), pc)
    pos = np.arange(L, dtype=np.int64) - padded_starts[sid]
    src_sorted = cstart[sid] + np.minimum(pos, counts[sid] - 1)
    perm = order[src_sorted]                  # padded stream -> signal row

    # 16 chunks at segment boundaries
    targets = (L * np.arange(1, 16, dtype=np.int64)) // 16
    split_segs = np.searchsorted(padded_starts, targets, side="left")
    seg_bounds = np.concatenate([[0], split_segs, [S]])
    seg_bounds = np.maximum.accumulate(seg_bounds)  # ensure monotone
    slot_bounds = padded_starts[seg_bounds]

    lh_real = np.diff(slot_bounds)
    l_half = int(-(-int(lh_real.max()) // TF) * TF)
    g_half = l_half // W
    ntiles = l_half // TF

    sig_t = np.ascontiguousarray(sig.T)       # [64, N]
    in_maps = []
    plans = []   # per half: (core, rows_lo, s_lo, s_hi, base_slot)
    for c in range(NCORES):
        X = np.empty((128, l_half), np.float32)
        M = np.empty((4, g_half), np.float32)
        for h in range(2):
            k = 2 * c + h
            s_lo, s_hi = int(seg_bounds[k]), int(seg_bounds[k + 1])
            b0, b1 = int(slot_bounds[k]), int(slot_bounds[k + 1])
            hperm = perm[b0:b1]
            if len(hperm) < l_half:
                pad_src = hperm[-1] if len(hperm) else 0
                hperm = np.concatenate(
                    [hperm, np.full(l_half - len(hperm), pad_src, np.int64)])
            X[64 * h:64 * (h + 1), :] = sig_t[:, hperm]
            mrow = np.zeros(g_half, np.float32)
            starts_local = (padded_starts[s_lo:s_hi] - b0) // W
            mrow[starts_local[(pc[s_lo:s_hi] > 0)]] = NEG
            mrow[(b1 - b0) // W:] = NEG       # dummy tail groups: isolate
            M[2 * h:2 * (h + 1), :] = mrow[None, :]
            plans.append((c, 64 * h, s_lo, s_hi, b0))
        in_maps.append({"x": X, "m": M})
    return in_maps, plans, padded_starts, pc, l_half, g_half, ntiles


def kernel(signal, cell_idx, num_segments):
    from concourse.bass_utils import run_bass_kernel_spmd

    sig = np.asarray(signal, dtype=np.float32)
    idx = np.asarray(cell_idx).astype(np.int64)
    S = int(num_segments)

    in_maps, plans, padded_starts, pc, l_half, g_half, ntiles = _preprocess(sig, idx, S)

    key = (l_half, g_half, ntiles)
    if key not in _nc_cache:
        _nc_cache[key] = _build_nc(l_half, g_half, ntiles)
    nc = _nc_cache[key]

    res = run_bass_kernel_spmd(nc, in_maps, core_ids=list(range(NCORES)))

    out = np.full((S, sig.shape[1]), -np.inf, np.float32)
    for (c, r0, s_lo, s_hi, b0) in plans:
        if s_hi <= s_lo:
            continue
        scan = res.results[c]["scan"][r0:r0 + 64]          # [64, g_half]
        nz = pc[s_lo:s_hi] > 0
        ends_local = (padded_starts[s_lo + 1:s_hi + 1] - b0) // W - 1
        out[np.arange(s_lo, s_hi)[nz]] = scan[:, ends_local[nz]].T
    return out
